# revision 10
# baseline (speedup 1.0000x reference)
"""Trainium2 Bass kernel for nn_PhysicsEngine (protein-ligand energy).

Strategy
--------
Data-parallel over batch B=8 across the 8 NeuronCores (one batch per core).
Per core the [NL=128, NP=8192] pairwise computation is restructured as:

  * TensorE matmuls produce all bilinear "planes" from small per-atom
    feature vectors:  C = dist^2 + sigma^2, U = dist^2, V = kv*sigma,
    Q = 83.015*qL*qP, E = -2.5*ccL*ccP.  Features are single fp16 rows
    (~5e-4 relative rounding, validated ~1e-3 end-to-end against the
    2e-2 gate) accumulated in fp32 PSUM; U is max-clamped at 1e-4
    before the log to guard against cancellation driving dist^2
    slightly negative.  The three planes of each phase run concurrently
    in separate 32-row PE groups (lhsT at base partitions 0/32/64).
  * All sqrt/rsqrt/reciprocal work is rewritten in log space so only
    Ln/Exp/Sigmoid ACT functions are needed (2 table sets):
        d      = Exp(0.5*Ln(max(U,1e-4)))
        rsq    = Exp(-0.5*Ln(C))             # 1/soft_dist
        r6     = Exp(6lnV+c) * Exp(-3lnC)    # ratio^6, two indep. exps
        hsa    = Sigmoid(-2*lnU + 4*ln4)     # 1/(1+(d/4)^4)
        mask   = Sigmoid(-2*d + 24)
    Tiny GpSimd-produced bias operands chain the ACT queue into
    [Ln,Exp]->[Sigmoid,Square] blocks to minimize table loads.
  * The softplus tail term delta = log1p(exp(-(vdw+10))) is reduced via
    first-order Taylor (error << 1):  SD = e^-10 * (sum(mask) -
    sum(vdw*mask)), reusing sums needed anyway.
  * VectorE does the remaining tensor*tensor work; global sums are fused
    into tensor_scalar / scalar_tensor_tensor / activation accum_out
    row-sums.  Host does the final tiny reduction and clamps in float64.

Host <-> device traffic is minimized: per core only 9 fp16 feature rows
([9, 8192]) plus the packed ligand-side weights ([66, 256] fp16) are
uploaded; the padded rhs layouts (all-ones rows, the PE-group replicas)
are assembled on-device with a handful of DMAs.  The per-ligand eps
scale is applied on the host to the per-row partial sums instead of
on-device.  The jitted PJRT executable is built once and cached, so
warm calls skip retracing (the warm call is ~1 proxy round trip).

The ratio = min(sigma/softdist, 5) clamp is provably inactive (ratio<=1),
and the soft upper clamp at 500 is an exact no-op in fp32 for the value
range here.
"""

import numpy as np
from contextlib import ExitStack

import concourse.bacc as bacc
import concourse.tile as tile
import concourse.mybir as mybir

AF = mybir.ActivationFunctionType
ALU = mybir.AluOpType
F32 = mybir.dt.float32
BF16 = mybir.dt.bfloat16
F16 = mybir.dt.float16
NPF16 = np.float16

# ---- problem constants (hardcoded; kernel.py must be self-contained) ----
B, NL, NP = 8, 128, 8192
N_CORES = 8
PROT_RADII = np.array([1.7, 1.55, 1.52, 1.8], dtype=np.float32)
T_GATE = float(np.float32(1.0) / (np.float32(1.0) + np.exp(np.float32(2.0))))
C_PAULI = 100.0 * T_GATE          # ~11.9202922
C_GHOST = 500.0
SQ_PAULI = float(np.sqrt(C_PAULI))
SQ_GHOST = float(np.sqrt(C_GHOST))
K_V = 0.6 * SQ_PAULI                          # V plane = K_V * sigma
R6_BIAS = float(-6.0 * np.log(K_V))           # bias for sigma^6 exp
HSA_BIAS = float(4.0 * np.log(4.0))           # 5.545177444
EM10 = float(np.exp(np.float64(-10.0)))       # e^-10 for the SD Taylor term
U_FLOOR = 1e-4                                # dist^2 clamp before Ln

# ---- tiling parameters ----
W = 4096              # full-width plane ops (per pass)
NPASS = NP // W       # 2
CH = 1024             # PSUM chunk width (2 banks)
NCH = W // CH         # 4
HW_ = W // 2          # half width for phase D
# output columns per pass: S1a(2) S1b(2) PV(2) M(2) G(1) SH(NCH)
OBS = 9 + NCH

# single-row feature layout (rows in the 3 PE groups at 0/32/64)
KC, KU, KV, KQ, KE = 8, 5, 2, 1, 1
RPAD = 66             # rows in padded rhs/weight tensors (64 + KV)
NOUT = OBS * NPASS

# compact upload row indices (pin tensor, [9, NP] fp16)
#  0 n2Px  1 n2Py  2 n2Pz  3 Psq  4 rP  5 rP2  6 qP  7 xP0  8 ones

# table sets the activation-table chooser may use
_KEEP_SETS = {"natural_log_exp_and_others", "sigmoid_and_others"}

_CACHE = {}


def _build_program():
    """Build the (SPMD, per-core) Bass program once."""
    nc = bacc.Bacc("TRN2", target_bir_lowering=False, debug=False,
                   num_devices=N_CORES)

    pin_d = nc.dram_tensor("pin", [9, NP], F16, kind="ExternalInput").ap()
    sml_d = nc.dram_tensor("sml", [RPAD, 256], F16, kind="ExternalInput").ap()
    out_d = nc.dram_tensor("out", [128, NOUT], F32, kind="ExternalOutput").ap()

    with tile.TileContext(nc) as tc, ExitStack() as ctx:
        planes = ctx.enter_context(tc.tile_pool(name="planes", bufs=1))
        smalls = ctx.enter_context(tc.tile_pool(name="smalls", bufs=1))
        pads = ctx.enter_context(tc.tile_pool(name="pads", bufs=1))
        scratch = ctx.enter_context(tc.tile_pool(name="scratch", bufs=2))
        psA = ctx.enter_context(tc.tile_pool(name="psA", bufs=1, space="PSUM"))

        wsb = smalls.tile([RPAD, 256], F16, name="wsb")
        nc.sync.dma_start(wsb[:], sml_d[:])
        out_sb = smalls.tile([128, NOUT], F32, name="out_sb")
        nc.gpsimd.memset(out_sb[:], 0.0)

        # persistent rhs tiles; all-ones rows DMAed once from pin row 8
        # (memset can't target unaligned partition bases), data rows
        # re-DMAed per pass from the compact pin tensor
        rpadA = pads.tile([RPAD, W], F16, name="rpadA")
        rpadC = pads.tile([RPAD, W], F16, name="rpadC")
        for pr in (6, 7, 36, 65):
            nc.sync.dma_start(rpadA[pr:pr + 1, :], pin_d[8:9, 0:W])
        nc.sync.dma_start(rpadC[33:34, :], pin_d[8:9, 0:W])

        # per-pass data DMAs: (dst_tile, dst_row_start, pin_row_start, n)
        _DMAS = (
            (0, 0, 0, 6),    # C rows: n2Px n2Py n2Pz Psq rP rP2
            (0, 32, 0, 4),   # U rows: n2Px n2Py n2Pz Psq
            (0, 64, 4, 1),   # V row: rP
            (1, 0, 6, 1),    # Q row: qP
            (1, 32, 4, 1),   # V2 row: rP
            (1, 64, 7, 1),   # E row: xP0
        )

        _consts = {}

        def cb(v):
            v = float(v)
            if v not in _consts:
                t = smalls.tile([128, 1], F32, name=f"cst{len(_consts)}")
                nc.gpsimd.memset(t[:], v)
                _consts[v] = t
            return _consts[v][:]

        def dyn_bias(nm, src, v):
            """[128,1] bias holding constant v, data-dependent on src (an AP);
            used to order the ACT queue into table-set blocks."""
            t = smalls.tile([128, 1], F32, name=nm)
            nc.gpsimd.tensor_scalar(t[:], src, 0.0, float(v),
                                    op0=ALU.mult, op1=ALU.add)
            return t[:]

        def plane(nm, dt=F32, **kw):
            return planes.tile([128, W], dt, name=nm, tag=nm, **kw)

        hsa_prev = None
        for p in range(NPASS):
            g0 = p * W
            ob = OBS * p
            last = p == NPASS - 1

            gh = slice(g0, g0 + W)
            for dst, dr, sr, n in _DMAS:
                t = rpadA if dst == 0 else rpadC
                nc.sync.dma_start(t[dr:dr + n, :], pin_d[sr:sr + n, gh])

            # ACT-order chaining: this pass's Ln ops wait on last pass's hsa
            if hsa_prev is None:
                b_lnU, b_ln0 = cb(1e-8), cb(0.0)
            else:
                b_lnU = dyn_bias(f"blnU{p}", hsa_prev, 1e-8)
                b_ln0 = dyn_bias(f"bln0{p}", hsa_prev, 0.0)

            # ---------- phase A: packed matmuls -> Ln evacuations ----------
            lnU = plane("lnU")
            lnC = plane("lnC")
            lnV = plane("lnV")
            for i in range(NCH):
                sl = slice(i * CH, (i + 1) * CH)
                C_ps = psA.tile([128, CH], F32, name="C_ps", tag="p0", bufs=2)
                U_ps = psA.tile([128, CH], F32, name="U_ps", tag="p1")
                V_ps = psA.tile([128, CH], F32, name="V_ps", tag="p2")
                for h in range(CH // 512):
                    ms = slice(h * 512, (h + 1) * 512)
                    rs = slice(i * CH + h * 512, i * CH + (h + 1) * 512)
                    nc.tensor.matmul(C_ps[:, ms], wsb[0:KC, 0:128],
                                     rpadA[0:KC, rs], start=True, stop=True)
                    nc.tensor.matmul(U_ps[:, ms], wsb[32:32 + KU, 0:128],
                                     rpadA[32:32 + KU, rs], start=True, stop=True)
                    nc.tensor.matmul(V_ps[:, ms], wsb[64:64 + KV, 0:128],
                                     rpadA[64:64 + KV, rs], start=True, stop=True)
                # clamp U at the floor before the log (fp16 geometry rounding
                # can push dist^2 for near-contact pairs slightly negative)
                Ucl = scratch.tile([128, CH], F32, name="ucl", tag="ucl")
                nc.vector.tensor_scalar(Ucl[:], U_ps[:], U_FLOOR, None,
                                        op0=ALU.max)
                nc.scalar.activation(lnU[:, sl], Ucl[:], AF.Ln, bias=b_lnU)
                nc.scalar.activation(lnC[:, sl], C_ps[:], AF.Ln, bias=b_ln0)
                nc.scalar.activation(lnV[:, sl], V_ps[:], AF.Ln, bias=b_ln0)

            # ---------- phase B: full-width log-space math ----------
            # r6 = sigma^6/C^3 via two independent exps, emitted first so the
            # DVE r6-chain starts while ACT continues with d/rsq
            if not last:
                b_e1 = cb(R6_BIAS)
                e1 = plane("e1", BF16)
                e2 = plane("e2", BF16)
                for h in range(2):
                    hs = slice(h * HW_, (h + 1) * HW_)
                    nc.scalar.activation(e1[:, hs], lnV[:, hs], AF.Exp,
                                         bias=b_e1, scale=6.0)
                    nc.scalar.activation(e2[:, hs], lnC[:, hs], AF.Exp,
                                         bias=cb(0.0), scale=-3.0)
            d = plane("d_pl")
            rsq = plane("rsq", BF16)
            for h in range(2):
                hs = slice(h * HW_, (h + 1) * HW_)
                nc.scalar.activation(d[:, hs], lnU[:, hs], AF.Exp,
                                     bias=cb(0.0), scale=0.5)
                nc.scalar.activation(rsq[:, hs], lnC[:, hs], AF.Exp,
                                     bias=cb(0.0), scale=-0.5)

            def emit_sigmoids(bm, bh):
                m = plane("mask", BF16)
                hh = plane("hsa", BF16)
                for h in range(2):
                    hs = slice(h * HW_, (h + 1) * HW_)
                    nc.scalar.activation(m[:, hs], d[:, hs], AF.Sigmoid,
                                         bias=bm, scale=-2.0)
                    nc.scalar.activation(hh[:, hs], lnU[:, hs], AF.Sigmoid,
                                         bias=bh, scale=-2.0)
                return m, hh

            if last:
                # tail pass: run sigmoids early (extra table loads are
                # cheaper than leaving DVE unfed at the end)
                b_mask = dyn_bias(f"bmask{p}", d[:, 0:1], 24.0)
                b_hsa = dyn_bias(f"bhsa{p}", d[:, 0:1], HSA_BIAS)
                mask, hsa = emit_sigmoids(b_mask, b_hsa)
                b_e1 = dyn_bias(f"be1{p}", mask[:, 0:1], R6_BIAS)
                e1 = plane("e1", BF16)
                nc.scalar.activation(e1[:], lnV[:], AF.Exp, bias=b_e1, scale=6.0)
                e2 = plane("e2", BF16)
                nc.scalar.activation(e2[:], lnC[:], AF.Exp, bias=cb(0.0),
                                     scale=-3.0)
            r6 = plane("r6", BF16)
            r6m1 = plane("tmp1", BF16)
            prod = plane("prod", BF16)
            for h in range(2):
                hs = slice(h * HW_, (h + 1) * HW_)
                nc.vector.tensor_tensor(r6[:, hs], e1[:, hs], e2[:, hs],
                                        op=ALU.mult)
                nc.vector.tensor_scalar(r6m1[:, hs], r6[:, hs], -1.0, None,
                                        op0=ALU.add)
                nc.vector.tensor_tensor(prod[:, hs], r6[:, hs], r6m1[:, hs],
                                        op=ALU.mult)

            if not last:
                b_mask = dyn_bias(f"bmask{p}", prod[:, 0:1], 24.0)
                b_hsa = dyn_bias(f"bhsa{p}", prod[:, 0:1], HSA_BIAS)
                mask, hsa = emit_sigmoids(b_mask, b_hsa)
            hsa_prev = hsa[:, 0:1]
            hm = plane("hm", BF16)
            for h in range(2):
                hs = slice(h * HW_, (h + 1) * HW_)
                nc.vector.tensor_tensor(hm[:, hs], hsa[:, hs], mask[:, hs],
                                        op=ALU.mult)

            # ghost: grm = -sqrt(500)*min(d, 0.5); g2 = (grm + c)^2, c chosen
            # so the bf16-rounded zero cancels exactly
            grm = planes.tile([128, W], BF16, name="grm", tag="tmp1")
            nc.vector.tensor_scalar(
                grm[:], d[:], 0.5, -SQ_GHOST, op0=ALU.min, op1=ALU.mult)
            gz = float(np.float32(0.5) * np.float32(-SQ_GHOST))
            import ml_dtypes as _mld
            b_g2 = dyn_bias(f"bg2{p}", hsa[:, 0:1],
                            -float(np.float32(_mld.bfloat16(gz))))
            g2 = plane("g2", BF16)
            nc.scalar.activation(g2[:], grm[:], AF.Square, bias=b_g2, scale=1.0,
                                 accum_out=out_sb[:, ob + 8: ob + 9])

            # ---------- phase C: chunked PSUM-consuming products ----------
            eelp = plane("eelp", BF16)
            ovin = plane("ovin", BF16)
            for i in range(NCH):
                sl = slice(i * CH, (i + 1) * CH)
                Q_ps = psA.tile([128, CH], F32, name="Q_ps", tag="p0", bufs=2)
                V2_ps = psA.tile([128, CH], F32, name="V2_ps", tag="p1")
                E_ps = psA.tile([128, CH], F32, name="E_ps", tag="p2")
                for h in range(CH // 512):
                    ms = slice(h * 512, (h + 1) * 512)
                    rs = slice(i * CH + h * 512, i * CH + (h + 1) * 512)
                    nc.tensor.matmul(Q_ps[:, ms], wsb[0:KQ, 128:256],
                                     rpadC[0:KQ, rs], start=True, stop=True)
                    nc.tensor.matmul(V2_ps[:, ms], wsb[32:32 + KV, 128:256],
                                     rpadC[32:32 + KV, rs], start=True, stop=True)
                    nc.tensor.matmul(E_ps[:, ms], wsb[64:64 + KE, 128:256],
                                     rpadC[64:64 + KE, rs], start=True, stop=True)
                # e_el = Q * rsq
                nc.vector.tensor_tensor(eelp[:, sl], Q_ps[:], rsq[:, sl],
                                        op=ALU.mult)
                # ovin = K_V*sigma - sqrt(C_PAULI)*d
                nc.vector.scalar_tensor_tensor(
                    ovin[:, sl], d[:, sl], -SQ_PAULI, V2_ps[:],
                    op0=ALU.mult, op1=ALU.add)
                # SH[:, chunk] = sum(hm * E)
                hsc = scratch.tile([128, CH], BF16, name="hsc", tag="hsc")
                nc.vector.scalar_tensor_tensor(
                    hsc[:], hm[:, sl], 0.0, E_ps[:], op0=ALU.add, op1=ALU.mult,
                    accum_out=out_sb[:, ob + 9 + i: ob + 10 + i])

            # ---------- phase D: reductions in 2048-halves ----------
            for h in range(2):
                hs = slice(h * HW_, (h + 1) * HW_)
                s1 = planes.tile([128, HW_], BF16, name="dveout",
                                 tag="dveout", bufs=2)
                nc.vector.tensor_tensor(s1[:], eelp[:, hs], mask[:, hs],
                                        op=ALU.mult)
                s1b = planes.tile([128, HW_], BF16, name="dveout",
                                  tag="dveout", bufs=2)
                nc.vector.tensor_scalar(
                    s1b[:], s1[:], 1.0, 0.0, op0=ALU.mult, op1=ALU.add,
                    accum_out=out_sb[:, ob + h: ob + h + 1])
                s2 = planes.tile([128, HW_], BF16, name="dveout",
                                 tag="dveout", bufs=2)
                nc.vector.tensor_tensor(s2[:], prod[:, hs], mask[:, hs],
                                        op=ALU.mult)
                s2b = planes.tile([128, HW_], BF16, name="dveout",
                                  tag="dveout", bufs=2)
                nc.vector.tensor_scalar(
                    s2b[:], s2[:], 1.0, 0.0, op0=ALU.mult, op1=ALU.add,
                    accum_out=out_sb[:, ob + 2 + h: ob + 3 + h])
                # pauli: relu(ovin)^2 = (ovin max 0)*ovin, fused row-sum
                s3 = planes.tile([128, HW_], BF16, name="dveout",
                                 tag="dveout", bufs=2)
                nc.vector.scalar_tensor_tensor(
                    s3[:], ovin[:, hs], 0.0, ovin[:, hs], op0=ALU.max,
                    op1=ALU.mult, accum_out=out_sb[:, ob + 4 + h: ob + 5 + h])
                # M = sum(mask) for the softplus Taylor term
                mby = planes.tile([128, HW_], BF16, name="dveout",
                                  tag="dveout", bufs=2)
                nc.vector.tensor_scalar(
                    mby[:], mask[:, hs], 1.0, 0.0, op0=ALU.mult, op1=ALU.add,
                    accum_out=out_sb[:, ob + 6 + h: ob + 7 + h])

        nc.sync.dma_start(out_d[:], out_sb[:])

    # Restrict the activation-table chooser to two sets (indices preserved;
    # contents of the others emptied) so Ln/Exp share one table and
    # Sigmoid/Square the other.
    import concourse.hw_specs as hw_specs
    _orig = bacc.get_activation_tables
    def _filtered(arch):
        full = hw_specs.get_activation_tables(arch)
        return {k: (v if k in _KEEP_SETS else set()) for k, v in full.items()}
    bacc.get_activation_tables = _filtered
    try:
        nc.compile()
    finally:
        bacc.get_activation_tables = _orig
    return nc


def _make_runner():
    """Compile the program and build a cached jitted PJRT callable.

    Replicates concourse.bass2jax.run_bass_via_pjrt's lowering, but
    hoists the jax.jit(shard_map(...)) construction out of the per-call
    path so warm calls skip retracing/relowering (~250 ms/call saved)."""
    import jax
    from jax.sharding import Mesh, PartitionSpec
    from jax.experimental.shard_map import shard_map
    from concourse.bass2jax import (
        install_neuronx_cc_hook, _bass_exec_p, partition_id_tensor)

    nc = _build_program()
    install_neuronx_cc_hook()

    partition_name = (nc.partition_id_tensor.name
                      if nc.partition_id_tensor else None)
    in_names, out_names, out_avals, zero_shapes = [], [], [], []
    for alloc in nc.m.functions[0].allocations:
        if not isinstance(alloc, mybir.MemoryLocationSet):
            continue
        name = alloc.memorylocations[0].name
        if alloc.kind == "ExternalInput":
            if name != partition_name:
                in_names.append(name)
        elif alloc.kind == "ExternalOutput":
            shape = tuple(alloc.tensor_shape)
            dtype = mybir.dt.np(alloc.dtype)
            out_names.append(name)
            out_avals.append(jax.core.ShapedArray(shape, dtype))
            zero_shapes.append((shape, dtype))
    n_params = len(in_names)
    n_outs = len(out_avals)
    in_names_full = list(in_names) + out_names + (
        [partition_name] if partition_name else [])
    donate = tuple(range(n_params, n_params + n_outs))

    def _body(*args):
        operands = list(args)
        if partition_name is not None:
            operands.append(partition_id_tensor())
        outs = _bass_exec_p.bind(
            *operands, out_avals=tuple(out_avals),
            in_names=tuple(in_names_full), out_names=tuple(out_names),
            lowering_input_output_aliases=(), sim_require_finite=True,
            sim_require_nnan=True, nc=nc)
        return tuple(outs)

    devices = jax.devices()[:N_CORES]
    mesh = Mesh(np.asarray(devices), ("core",))
    in_specs = (PartitionSpec("core"),) * (n_params + n_outs)
    out_specs = (PartitionSpec("core"),) * len(out_names)
    sharded = jax.jit(
        shard_map(_body, mesh=mesh, in_specs=in_specs, out_specs=out_specs,
                  check_rep=False),
        donate_argnums=donate, keep_unused=True)

    return dict(nc=nc, sharded=sharded, in_names=in_names,
                out_names=out_names, out_avals=out_avals,
                zero_shapes=zero_shapes)


def _prep_pin(pos_P, q_P, x_P):
    """All-batch compact protein-side rows: [B*9, NP] fp16."""
    P = pos_P.astype(np.float32)                      # [B, NP, 3]
    rP = x_P.astype(np.float32) @ PROT_RADII          # [B, NP]

    pin = np.empty((B, 9, NP), NPF16)
    pin[:, 0] = -2.0 * P[..., 0]
    pin[:, 1] = -2.0 * P[..., 1]
    pin[:, 2] = -2.0 * P[..., 2]
    pin[:, 3] = (P * P).sum(-1)
    pin[:, 4] = rP
    pin[:, 5] = rP * rP
    pin[:, 6] = q_P
    pin[:, 7] = x_P[..., 0]
    pin[:, 8] = NPF16(1.0)
    return pin.reshape(B * 9, NP)


def _prep_sml(pos_L, q_L, x_L, vdw_radii):
    """All-batch ligand-side packed weights: [B*RPAD, 256] fp16.

    Columns 0:128 hold the phase-A weights (C/U/V groups at rows
    0/32/64), columns 128:256 the phase-C weights (Q/V2/E groups)."""
    L = pos_L.astype(np.float32)                      # [B, 128, 3]
    rL = x_L.astype(np.float32) @ vdw_radii.astype(np.float32)   # [B, 128]

    sml = np.zeros((B, RPAD, 256), NPF16)
    wA = sml[:, :, 0:128]
    wC = sml[:, :, 128:256]
    # C rows: [Lx, Ly, Lz, 1, 2rL, 1, Lsq, rL2]
    wA[:, 0] = L[..., 0]
    wA[:, 1] = L[..., 1]
    wA[:, 2] = L[..., 2]
    wA[:, 3] = 1.0
    wA[:, 4] = 2.0 * rL
    wA[:, 5] = 1.0
    wA[:, 6] = (L * L).sum(-1)
    wA[:, 7] = rL * rL
    # U rows: [Lx, Ly, Lz, 1, Lsq]
    wA[:, 32] = L[..., 0]
    wA[:, 33] = L[..., 1]
    wA[:, 34] = L[..., 2]
    wA[:, 35] = 1.0
    wA[:, 36] = (L * L).sum(-1)
    # V rows: [K_V, K_V*rL]
    wA[:, 64] = NPF16(K_V)
    wA[:, 65] = np.float32(K_V) * rL
    # Q row; V2 rows; E row
    wC[:, 0] = np.float32(332.06 / 4.0) * q_L.astype(np.float32)
    wC[:, 32] = NPF16(K_V)
    wC[:, 33] = np.float32(K_V) * rL
    wC[:, 64] = np.float32(-2.5) * x_L[..., 0].astype(np.float32)
    return sml.reshape(B * RPAD, 256)


def _finish(core_out, eps4):
    """core_out: [128, OBS*NPASS] f32 partial sums for one batch;
    eps4: [128] f64 per-ligand vdw scale applied host-side.

    Columns per pass: 0,1 S1a halves; 2,3 S1b halves; 4,5 PV halves;
    6,7 M halves; 8 G; 9.. SH chunks."""
    o = core_out.astype(np.float64).reshape(128, NPASS, OBS)
    S1a = o[:, :, 0:2].sum()
    S1b = float(o[:, :, 2:4].sum(axis=(1, 2)) @ eps4)
    PV = o[:, :, 4:6].sum()
    M = o[:, :, 6:8].sum()
    G = o[:, :, 8].sum()
    SH = o[:, :, 9:OBS].sum()
    S1 = S1a + S1b
    SD = EM10 * (M - S1b)
    pg = PV + G
    e_soft = S1 + SD
    e_raw = e_soft + SH + pg
    e_hard = min(pg, 10000.0)
    log_soft = S1 + SH
    e_soft_final = min(max(log_soft, -500.0), 5000.0)
    log_energy = min(e_soft_final + e_hard, 1.0e6)
    return e_raw, e_hard, log_energy


def kernel(pos_L, pos_P, q_L, q_P, x_L, x_P, vdw_radii, epsilon, _res_hook=None):
    if "st" not in _CACHE:
        _CACHE["st"] = _make_runner()
    st = _CACHE["st"]

    pin_all = _prep_pin(pos_P, q_P, x_P)
    sml_all = _prep_sml(pos_L, q_L, x_L, vdw_radii)
    by_name = {"pin": pin_all, "sml": sml_all}
    concat_in = [by_name[n] for n in st["in_names"]]
    concat_zeros = [np.zeros((N_CORES * s[0], *s[1:]), d)
                    for s, d in st["zero_shapes"]]

    out_arrs = st["sharded"](*concat_in, *concat_zeros)
    oi = st["out_names"].index("out")
    full = np.asarray(out_arrs[oi]).reshape(
        N_CORES, *st["out_avals"][oi].shape)

    epsL = np.maximum(x_L.astype(np.float32) @ epsilon.astype(np.float32), 0.0)
    eps4 = (4.0 * np.sqrt(epsL * np.float32(0.15) + np.float32(1e-8))
            ).astype(np.float64)                      # [B, 128]

    e_raw = np.empty(B, dtype=np.float32)
    e_hard = np.empty(B, dtype=np.float32)
    log_e = np.empty(B, dtype=np.float32)
    for b in range(B):
        r, h, l = _finish(full[b], eps4[b])
        e_raw[b], e_hard[b], log_e[b] = r, h, l
    return e_raw, e_hard, log_e
